# revision 1
# baseline (speedup 1.0000x reference)
"""TRN2 Bass kernel for 2-level hierarchical MoE (nn_MoELayer_47914655154654).

Math (per level, exactly equivalent to the reference):
  probs = softmax(x @ Wr); top-2 binary mask m; w = probs * m
  For non-selected experts the masked input is 0, so their FFN output is the
  per-expert constant c_e = relu(b1_e) @ W2_e + b2_e. Hence
    out = sum_e w_e * U_e(x)  +  w @ (b2 - C)  +  probs @ C
  with U_e(x) = relu(x @ W1_e + b1_e) @ W2_e (no b2) and C = [c_e].

Architecture: routing, token->expert sorting and the (tiny) affine terms run
on the host in f32; the device runs two launches of a pure batched-expert FFN
over pre-sorted token slots — top-2 sparse FLOPs only. Each launch is an SPMD
program of uniform single-expert segments (mostly 512 tokens wide); WHICH
expert a segment serves is input data (per-core gathered weight arrays), so
arbitrary expert imbalance packs with <10% padding and no per-core program
divergence. Segment shapes are derived from the realized counts and compiled
kernels are cached per shape signature.

Numerics: a single routing flip fails an absmax gate, so the level-0 h-matmul
(h0 feeds the level-1 router; min L1 top2/3 logit gap is ~1.6e-5 abs) runs as
a 3-pass split-bf16 matmul (hi/lo mantissa split, ~1e-5 err). The level-0
y-matmul is 1-pass bf16: its rounding error is cancelled where it matters by
a low-rank correction dl = h_lo @ (W2@Wr1) + h_hi @ (W2_lo@Wr1) computed on
device ([2048,8] V-matrices host-precomputed) and added to the L1 router
logits on the host. Level-1 FFN is plain bf16 (values only). Routing and
combine weights stay f32 end to end. Measured vs reference: rel err 4.1e-3,
absmax/scale 4.3e-3, zero routing flips.

Sharding: data parallel — each core processes its share of (token, expert)
slots with replicated weights; activations are d-major on chip.
"""
import numpy as np
import ml_dtypes

import concourse.bass as bass
import concourse.tile as tile
from concourse import bacc, mybir
from concourse.bass_utils import run_bass_kernel_spmd

F32 = mybir.dt.float32
BF16 = mybir.dt.bfloat16
AF = mybir.ActivationFunctionType
ALU = mybir.AluOpType
BF = ml_dtypes.bfloat16

P = 128
D = 512
DFF = 2048
E0, E1 = 4, 8
NCORES = 8
KD = D // P           # 4
NFF = DFF // P        # 16
ND = D // P           # 4
MAXN = 512            # max matmul free dim / PSUM bank width

_CACHE = {}
_LAST_IN_MAPS = {}


def _chunks(cap):
    n = -(-cap // MAXN)
    base = cap // n
    szs = [base + (1 if i < cap - base * n else 0) for i in range(n)]
    return szs


def _build_ffn(E, caps, split):
    """Batched-expert FFN launch: for each expert e, tokens in its slot range
    get y = w_slot * (relu(x @ W1_e + b1_e) @ W2_e), d-major in/out."""
    key = ("ffn", E, tuple(caps), split)
    if key in _CACHE:
        return _CACHE[key]
    NTOT = sum(caps)
    nc = bacc.Bacc("TRN2", target_bir_lowering=False, debug=False,
                   num_devices=NCORES)
    d = {}
    d["xh"] = nc.dram_tensor("xh", [D, NTOT], BF16, kind="ExternalInput").ap()
    d["w1h"] = nc.dram_tensor("w1h", [E, D, DFF], BF16,
                              kind="ExternalInput").ap()
    d["w2h"] = nc.dram_tensor("w2h", [E, DFF, D], BF16,
                              kind="ExternalInput").ap()
    if split:
        d["xl"] = nc.dram_tensor("xl", [D, NTOT], BF16,
                                 kind="ExternalInput").ap()
        d["w1l"] = nc.dram_tensor("w1l", [E, D, DFF], BF16,
                                  kind="ExternalInput").ap()
        d["vf"] = nc.dram_tensor("vf", [E, DFF, E1], BF16,
                                 kind="ExternalInput").ap()
        d["vl"] = nc.dram_tensor("vl", [E, DFF, E1], BF16,
                                 kind="ExternalInput").ap()
        dlT = nc.dram_tensor("dlT", [E1, NTOT], F32,
                             kind="ExternalOutput").ap()
    d["ws"] = nc.dram_tensor("ws", [1, NTOT], F32, kind="ExternalInput").ap()
    d["b1"] = nc.dram_tensor("b1", [E, DFF], F32, kind="ExternalInput").ap()
    outT = nc.dram_tensor("outT", [D, NTOT], F32, kind="ExternalOutput").ap()

    ts = bass.ts
    with tile.TileContext(nc) as tc:
        with tc.tile_pool(name="consts", bufs=1) as consts, \
             tc.tile_pool(name="xp", bufs=1) as xp, \
             tc.tile_pool(name="wpool", bufs=1) as wpool, \
             tc.tile_pool(name="hpool", bufs=2) as hpool, \
             tc.tile_pool(name="wbpool", bufs=2) as wbpool, \
             tc.tile_pool(name="outp", bufs=1) as outp, \
             tc.tile_pool(name="psh", bufs=4, space="PSUM") as psh, \
             tc.tile_pool(name="psy", bufs=2, space="PSUM") as psy:

            ones1 = consts.tile([1, P], F32, tag="ones1", name="ones1")
            nc.vector.memset(ones1[:], 1.0)
            b1s = consts.tile([P, E * NFF], F32, tag="b1s", name="b1s")
            nc.sync.dma_start(
                b1s[:].rearrange("p (e t) -> p e t", e=E),
                d["b1"].rearrange("e (t p) -> p e t", p=P))
            ws = consts.tile([1, NTOT], F32, tag="ws", name="ws")
            nc.sync.dma_start(ws[:], d["ws"])

            if split:
                dsb = outp.tile([E1, NTOT], F32, tag="dsb", name="dsb")

            off = 0
            for e in range(E):
                w1h = [wpool.tile([P, DFF], BF16, tag=f"w1h{k}",
                                  name=f"w1h{k}") for k in range(KD)]
                for k in range(KD):
                    nc.sync.dma_start(w1h[k][:], d["w1h"][e, ts(k, P), :])
                if split:
                    w1l = [wpool.tile([P, DFF], BF16, tag=f"w1l{k}",
                                      name=f"w1l{k}") for k in range(KD)]
                    for k in range(KD):
                        nc.sync.dma_start(w1l[k][:], d["w1l"][e, ts(k, P), :])
                w2h = [wpool.tile([P, D], BF16, tag=f"w2h{f}",
                                  name=f"w2h{f}") for f in range(NFF)]
                for f in range(NFF):
                    nc.sync.dma_start(w2h[f][:], d["w2h"][e, ts(f, P), :])
                if split:
                    vft = wpool.tile([P, NFF * E1], BF16, tag="vft",
                                     name="vft")
                    nc.sync.dma_start(
                        vft[:].rearrange("p (f c) -> p f c", f=NFF),
                        d["vf"][e].rearrange("(f p) c -> p f c", p=P))
                    vlt = wpool.tile([P, NFF * E1], BF16, tag="vlt",
                                     name="vlt")
                    nc.sync.dma_start(
                        vlt[:].rearrange("p (f c) -> p f c", f=NFF),
                        d["vl"][e].rearrange("(f p) c -> p f c", p=P))

                for N in _chunks(caps[e]):
                    xh = [xp.tile([P, MAXN], BF16, tag=f"xh{k}",
                                  name=f"xh{k}", bufs=3) for k in range(KD)]
                    for k in range(KD):
                        nc.sync.dma_start(xh[k][:, :N],
                                          d["xh"][ts(k, P), off:off + N])
                    if split:
                        xl = [xp.tile([P, MAXN], BF16, tag=f"xl{k}",
                                      name=f"xl{k}", bufs=3)
                              for k in range(KD)]
                        for k in range(KD):
                            nc.sync.dma_start(xl[k][:, :N],
                                              d["xl"][ts(k, P), off:off + N])
                    # broadcast w over partitions: [1,N] -> [128,N]
                    wb_ps = psh.tile([P, MAXN], F32, tag="h", name="wb_ps")
                    nc.tensor.matmul(wb_ps[:, :N], ones1[:],
                                     ws[0:1, off:off + N],
                                     start=True, stop=True)
                    wb = wbpool.tile([P, MAXN], F32, tag="wb", name="wb")
                    nc.scalar.copy(wb[:, :N], wb_ps[:, :N])

                    hhi, hlo = [], []
                    for f in range(NFF):
                        h_ps = psh.tile([P, MAXN], F32, tag="h", name="h_ps")
                        for k in range(KD):
                            nc.tensor.matmul(
                                h_ps[:, :N], w1h[k][:, ts(f, P)],
                                xh[k][:, :N],
                                start=(k == 0),
                                stop=(not split and k == KD - 1))
                        if split:
                            for k in range(KD):
                                nc.tensor.matmul(
                                    h_ps[:, :N], w1h[k][:, ts(f, P)],
                                    xl[k][:, :N],
                                    start=False, stop=False)
                            for k in range(KD):
                                nc.tensor.matmul(
                                    h_ps[:, :N], w1l[k][:, ts(f, P)],
                                    xh[k][:, :N],
                                    start=False, stop=(k == KD - 1))
                        bias = b1s[:, e * NFF + f:e * NFF + f + 1]
                        if split:
                            hf = hpool.tile([P, MAXN], F32, tag="hf",
                                            name="hf")
                            nc.scalar.activation(hf[:, :N], h_ps[:, :N],
                                                 AF.Relu, bias=bias)
                            hh = hpool.tile([P, MAXN], BF16, tag=f"hh{f}",
                                            name=f"hh{f}")
                            nc.vector.tensor_copy(hh[:, :N], hf[:, :N])
                            hl = hpool.tile([P, MAXN], BF16, tag=f"hl{f}",
                                            name=f"hl{f}")
                            nc.vector.scalar_tensor_tensor(
                                hl[:, :N], hh[:, :N], -1.0, hf[:, :N],
                                ALU.mult, ALU.add)
                            hhi.append(hh)
                            hlo.append(hl)
                        else:
                            hh = hpool.tile([P, MAXN], BF16, tag=f"hh{f}",
                                            name=f"hh{f}")
                            nc.scalar.activation(hh[:, :N], h_ps[:, :N],
                                                 AF.Relu, bias=bias)
                            hhi.append(hh)

                    for dt in range(ND):
                        y_ps = psy.tile([P, MAXN], F32, tag="y", name="y_ps")
                        for f in range(NFF):
                            nc.tensor.matmul(y_ps[:, :N],
                                             w2h[f][:, ts(dt, P)],
                                             hhi[f][:, :N],
                                             start=(f == 0),
                                             stop=(f == NFF - 1))
                        # scale by w and write out
                        ot = outp.tile([P, MAXN], F32, tag=f"ot{dt}",
                                       name=f"ot{dt}", bufs=3)
                        nc.vector.tensor_mul(ot[:, :N], y_ps[:, :N],
                                             wb[:, :N])
                        nc.sync.dma_start(outT[ts(dt, P), off:off + N],
                                          ot[:, :N])
                    if split:
                        # low-rank router correction:
                        # dl = h_lo @ Vfull + h_hi @ Vlo   [E1, N]
                        dl_ps = psy.tile([P, MAXN], F32, tag="y",
                                         name="dl_ps")
                        for f in range(NFF):
                            nc.tensor.matmul(dl_ps[0:E1, :N],
                                             vft[:, ts(f, E1)],
                                             hlo[f][:, :N],
                                             start=(f == 0), stop=False)
                            nc.tensor.matmul(dl_ps[0:E1, :N],
                                             vlt[:, ts(f, E1)],
                                             hhi[f][:, :N],
                                             start=False,
                                             stop=(f == NFF - 1))
                        nc.scalar.copy(dsb[0:E1, off:off + N],
                                       dl_ps[0:E1, :N])
                    off += N

            if split:
                nc.sync.dma_start(dlT, dsb[:])

    nc.compile()
    _CACHE[key] = nc
    return nc


def _route(xf, Wr, logits=None):
    """f32 routing identical to the reference ordering."""
    if logits is None:
        logits = xf @ Wr
    idx = np.argsort(-logits, axis=-1, kind='stable')[:, :2]
    mx = logits.max(-1, keepdims=True)
    p = np.exp(logits - mx)
    p /= p.sum(-1, keepdims=True)
    m = np.zeros_like(p)
    np.put_along_axis(m, idx, 1.0, axis=-1)
    w = p * m
    return p, w, idx


def _make_slots(idx, w, E):
    """Pack (token, expert) pairs into per-core, per-expert slot ranges."""
    ntok = idx.shape[0]
    caps = []
    tok_lists = []
    for e in range(E):
        toks = np.nonzero((idx == e).any(-1))[0]
        tok_lists.append(toks)
        per_core = -(-len(toks) // NCORES)
        caps.append(max(32, -(-per_core // 32) * 32))
    NTOT = sum(caps)
    perm = np.zeros((NCORES, NTOT), np.int64)
    wslot = np.zeros((NCORES, NTOT), np.float32)
    gid = np.zeros((ntok, 2), np.int64)
    gw = np.zeros((ntok, 2), np.float32)
    gcnt = np.zeros(ntok, np.int64)
    offs = np.cumsum([0] + caps[:-1])
    for e in range(E):
        toks = tok_lists[e]
        n = len(toks)
        base = n // NCORES
        rem = n - base * NCORES
        start = 0
        for c in range(NCORES):
            sz = base + (1 if c < rem else 0)
            t = toks[start:start + sz]
            start += sz
            sl = offs[e] + np.arange(sz)
            perm[c, sl] = t
            wslot[c, sl] = w[t, e]
            g = c * NTOT + sl
            gid[t, gcnt[t]] = g
            gw[t, gcnt[t]] = w[t, e]
            gcnt[t] += 1
    assert (gcnt == 2).all(), "every token must hit exactly two experts"
    return caps, NTOT, perm, wslot, gid, gw




def _build_seg(segs, split):
    """Segment-packed FFN launch: every core runs the same list of
    single-expert segments; which expert each segment serves is pure input
    data (per-core gathered weight arrays). With split=True the first matmul
    runs as 3-pass split-bf16 and the low-rank router correction dl is
    emitted (level 0); otherwise plain bf16 (level 1)."""
    key = ("seg", tuple(segs), split)
    if key in _CACHE:
        return _CACHE[key]
    NSEG = len(segs)
    NTOT = sum(segs)
    nc = bacc.Bacc("TRN2", target_bir_lowering=False, debug=False,
                   num_devices=NCORES)
    xh_d = nc.dram_tensor("xh", [D, NTOT], BF16, kind="ExternalInput").ap()
    w1_d = nc.dram_tensor("w1s", [NSEG, D, DFF], BF16,
                          kind="ExternalInput").ap()
    w2_d = nc.dram_tensor("w2s", [NSEG, DFF, D], BF16,
                          kind="ExternalInput").ap()
    ws_d = nc.dram_tensor("ws", [1, NTOT], F32, kind="ExternalInput").ap()
    b1_d = nc.dram_tensor("b1s", [NSEG, DFF], F32, kind="ExternalInput").ap()
    outT = nc.dram_tensor("outT", [D, NTOT], F32, kind="ExternalOutput").ap()
    if split:
        xl_d = nc.dram_tensor("xl", [D, NTOT], BF16,
                              kind="ExternalInput").ap()
        w1l_d = nc.dram_tensor("w1ls", [NSEG, D, DFF], BF16,
                               kind="ExternalInput").ap()
        vf_d = nc.dram_tensor("vfs", [NSEG, DFF, E1], BF16,
                              kind="ExternalInput").ap()
        vl_d = nc.dram_tensor("vls", [NSEG, DFF, E1], BF16,
                              kind="ExternalInput").ap()
        dlT = nc.dram_tensor("dlT", [E1, NTOT], F32,
                             kind="ExternalOutput").ap()

    ts = bass.ts
    with tile.TileContext(nc) as tc:
        with tc.tile_pool(name="consts", bufs=1) as consts, \
             tc.tile_pool(name="xp", bufs=1) as xp, \
             tc.tile_pool(name="wpool", bufs=1 if split else 2) as wpool, \
             tc.tile_pool(name="hpool", bufs=2) as hpool, \
             tc.tile_pool(name="wbpool", bufs=2) as wbpool, \
             tc.tile_pool(name="outp", bufs=1) as outp, \
             tc.tile_pool(name="psh", bufs=4, space="PSUM") as psh, \
             tc.tile_pool(name="psy", bufs=2, space="PSUM") as psy:

            ones1 = consts.tile([1, P], F32, tag="ones1", name="ones1")
            nc.vector.memset(ones1[:], 1.0)
            b1s = consts.tile([P, NSEG * NFF], F32, tag="b1s", name="b1s")
            nc.sync.dma_start(
                b1s[:].rearrange("p (s t) -> p s t", s=NSEG),
                b1_d.rearrange("s (t p) -> p s t", p=P))
            ws = consts.tile([1, NTOT], F32, tag="ws", name="ws")
            nc.sync.dma_start(ws[:], ws_d)
            if split:
                dsb = outp.tile([E1, NTOT], F32, tag="dsb", name="dsb")

            off = 0
            for s, N in enumerate(segs):
                w1t = [wpool.tile([P, DFF], BF16, tag=f"w1k{k}",
                                  name=f"w1k{k}") for k in range(KD)]
                for k in range(KD):
                    nc.sync.dma_start(w1t[k][:], w1_d[s, ts(k, P), :])
                xh = [xp.tile([P, MAXN], BF16, tag=f"xh{k}", name=f"xh{k}",
                              bufs=3) for k in range(KD)]
                for k in range(KD):
                    nc.sync.dma_start(xh[k][:, :N],
                                      xh_d[ts(k, P), off:off + N])
                if split:
                    w1lt = [wpool.tile([P, DFF], BF16, tag=f"w1l{k}",
                                       name=f"w1l{k}") for k in range(KD)]
                    for k in range(KD):
                        nc.sync.dma_start(w1lt[k][:], w1l_d[s, ts(k, P), :])
                    xl = [xp.tile([P, MAXN], BF16, tag=f"xl{k}",
                                  name=f"xl{k}", bufs=3) for k in range(KD)]
                    for k in range(KD):
                        nc.sync.dma_start(xl[k][:, :N],
                                          xl_d[ts(k, P), off:off + N])
                w2t = [wpool.tile([P, D], BF16, tag=f"w2k{f}",
                                  name=f"w2k{f}") for f in range(NFF)]
                for f in range(NFF):
                    nc.sync.dma_start(w2t[f][:], w2_d[s, ts(f, P), :])
                if split:
                    vft = wpool.tile([P, NFF * E1], BF16, tag="vft",
                                     name="vft")
                    nc.sync.dma_start(
                        vft[:].rearrange("p (f c) -> p f c", f=NFF),
                        vf_d[s].rearrange("(f p) c -> p f c", p=P))
                    vlt = wpool.tile([P, NFF * E1], BF16, tag="vlt",
                                     name="vlt")
                    nc.sync.dma_start(
                        vlt[:].rearrange("p (f c) -> p f c", f=NFF),
                        vl_d[s].rearrange("(f p) c -> p f c", p=P))

                wb_ps = psh.tile([P, MAXN], F32, tag="h", name="wb_ps")
                nc.tensor.matmul(wb_ps[:, :N], ones1[:],
                                 ws[0:1, off:off + N], start=True, stop=True)
                wb = wbpool.tile([P, MAXN], F32, tag="wb", name="wb")
                nc.scalar.copy(wb[:, :N], wb_ps[:, :N])

                hhi, hlo = [], []
                for f in range(NFF):
                    h_ps = psh.tile([P, MAXN], F32, tag="h", name="h_ps")
                    for k in range(KD):
                        nc.tensor.matmul(h_ps[:, :N], w1t[k][:, ts(f, P)],
                                         xh[k][:, :N], start=(k == 0),
                                         stop=(not split and k == KD - 1))
                    if split:
                        for k in range(KD):
                            nc.tensor.matmul(h_ps[:, :N], w1t[k][:, ts(f, P)],
                                             xl[k][:, :N],
                                             start=False, stop=False)
                        for k in range(KD):
                            nc.tensor.matmul(h_ps[:, :N],
                                             w1lt[k][:, ts(f, P)],
                                             xh[k][:, :N],
                                             start=False, stop=(k == KD - 1))
                    bias = b1s[:, s * NFF + f:s * NFF + f + 1]
                    if split:
                        hf = hpool.tile([P, MAXN], F32, tag="hf", name="hf")
                        nc.scalar.activation(hf[:, :N], h_ps[:, :N],
                                             AF.Relu, bias=bias)
                        hh = hpool.tile([P, MAXN], BF16, tag=f"hh{f}",
                                        name=f"hh{f}")
                        nc.vector.tensor_copy(hh[:, :N], hf[:, :N])
                        hl = hpool.tile([P, MAXN], BF16, tag=f"hl{f}",
                                        name=f"hl{f}")
                        nc.vector.scalar_tensor_tensor(
                            hl[:, :N], hh[:, :N], -1.0, hf[:, :N],
                            ALU.mult, ALU.add)
                        hhi.append(hh)
                        hlo.append(hl)
                    else:
                        hh = hpool.tile([P, MAXN], BF16, tag=f"hh{f}",
                                        name=f"hh{f}")
                        nc.scalar.activation(hh[:, :N], h_ps[:, :N],
                                             AF.Relu, bias=bias)
                        hhi.append(hh)

                for dt in range(ND):
                    y_ps = psy.tile([P, MAXN], F32, tag="y", name="y_ps")
                    for f in range(NFF):
                        nc.tensor.matmul(y_ps[:, :N], w2t[f][:, ts(dt, P)],
                                         hhi[f][:, :N],
                                         start=(f == 0), stop=(f == NFF - 1))
                    ot = outp.tile([P, MAXN], F32, tag=f"ot{dt}",
                                   name=f"ot{dt}", bufs=3)
                    nc.vector.tensor_mul(ot[:, :N], y_ps[:, :N], wb[:, :N])
                    nc.sync.dma_start(outT[ts(dt, P), off:off + N],
                                      ot[:, :N])
                if split:
                    # low-rank router correction dl = h_lo@Vfull + h_hi@Vlo
                    dl_ps = psy.tile([P, MAXN], F32, tag="y", name="dl_ps")
                    for f in range(NFF):
                        nc.tensor.matmul(dl_ps[0:E1, :N], vft[:, ts(f, E1)],
                                         hlo[f][:, :N],
                                         start=(f == 0), stop=False)
                        nc.tensor.matmul(dl_ps[0:E1, :N], vlt[:, ts(f, E1)],
                                         hhi[f][:, :N],
                                         start=False, stop=(f == NFF - 1))
                    nc.scalar.copy(dsb[0:E1, off:off + N], dl_ps[0:E1, :N])
                off += N

            if split:
                nc.sync.dma_start(dlT, dsb[:])

    nc.compile()
    _CACHE[key] = nc
    return nc


def _pack_segments(idx, w, E):
    """Pack (token, expert) pairs into uniform per-core segment lists.
    Returns (segs, seg_expert [NCORES, NSEG], perm, wslot, gid, gw) or None
    if the fixed bin shape cannot hold the realized distribution."""
    ntok = idx.shape[0]
    tok_lists = [np.nonzero((idx == e).any(-1))[0] for e in range(E)]
    # chunks: (expert, ntokens, binsize)
    big, small = [], []
    order = np.argsort([-len(t) for t in tok_lists])
    for e in order:
        r = len(tok_lists[e])
        pos = 0
        while r > MAXN:
            big.append((e, pos, MAXN))
            pos += MAXN
            r -= MAXN
        if r == 0:
            continue
        if r <= 256:
            small.append((e, pos, r))
        else:
            big.append((e, pos, r))
    n512 = -(-len(big) // NCORES)
    n256 = -(-len(small) // NCORES)
    if small:
        small_sz = max(32, -(-max(r for _, _, r in small) // 32) * 32)
    else:
        small_sz = 0
    segs = [MAXN] * n512 + [small_sz] * n256
    while len(big) < n512 * NCORES:
        big.append((0, 0, 0))
    while len(small) < n256 * NCORES:
        small.append((0, 0, 0))
    NSEG = len(segs)
    NTOT = sum(segs)
    seg_expert = np.zeros((NCORES, NSEG), np.int64)
    perm = np.zeros((NCORES, NTOT), np.int64)
    wslot = np.zeros((NCORES, NTOT), np.float32)
    gid = np.zeros((ntok, 2), np.int64)
    gw = np.zeros((ntok, 2), np.float32)
    gcnt = np.zeros(ntok, np.int64)
    offs = np.cumsum([0] + segs[:-1])
    for c in range(NCORES):
        items = [big[c * n512 + i] for i in range(n512)] + \
                [small[c * n256 + i] for i in range(n256)]
        for s, (e, pos, n) in enumerate(items):
            seg_expert[c, s] = e
            if n == 0:
                continue
            t = tok_lists[e][pos:pos + n]
            sl = offs[s] + np.arange(n)
            perm[c, sl] = t
            wslot[c, sl] = w[t, e]
            g = c * NTOT + sl
            gid[t, gcnt[t]] = g
            gw[t, gcnt[t]] = w[t, e]
            gcnt[t] += 1
    if not (gcnt == 2).all():
        return None
    return segs, seg_expert, perm, wslot, gid, gw


def _run_level_seg(xf, x_dev_hi, x_dev_lo, Wr, W1, b1, W2, b2, split,
                   wm, logits=None):
    """One MoE level via segment packing. Returns (out, dlog) or None if the
    packing does not fit (caller falls back to the per-expert-caps path)."""
    E = Wr.shape[1]
    p, w, idx = _route(xf, Wr, logits)
    packed = _pack_segments(idx, w, E)
    if packed is None:
        return None
    segs, seg_expert, perm, wslot, gid, gw = packed
    nc = _build_seg(segs, split)
    in_maps = []
    for c in range(NCORES):
        se = seg_expert[c]
        m = {
            "xh": np.ascontiguousarray(x_dev_hi[perm[c]].T),
            "w1s": np.ascontiguousarray(wm["w1h"][se]),
            "w2s": np.ascontiguousarray(wm["w2h"][se]),
            "b1s": np.ascontiguousarray(wm["b1"][se]),
            "ws": wslot[c:c + 1],
        }
        if split:
            m["xl"] = np.ascontiguousarray(x_dev_lo[perm[c]].T)
            m["w1ls"] = np.ascontiguousarray(wm["w1l"][se])
            m["vfs"] = np.ascontiguousarray(wm["vf"][se])
            m["vls"] = np.ascontiguousarray(wm["vl"][se])
        in_maps.append(m)
    _LAST_IN_MAPS[("seg", tuple(segs), split)] = in_maps
    res = run_bass_kernel_spmd(nc, in_maps, core_ids=list(range(NCORES)))
    Y = np.concatenate([res.results[c]["outT"] for c in range(NCORES)],
                       axis=1)
    Cc = np.einsum('ef,efd->ed', np.maximum(b1, 0.0), W2) + b2
    out = p @ Cc + w @ (b2 - Cc)
    out += Y[:, gid[:, 0]].T
    out += Y[:, gid[:, 1]].T
    dlog = None
    if split:
        DL = np.concatenate([res.results[c]["dlT"] for c in range(NCORES)],
                            axis=1)
        dlog = (DL[:, gid[:, 0]].T * gw[:, 0:1]
                + DL[:, gid[:, 1]].T * gw[:, 1:2])
    return out, dlog


def _run_level(xf, x_dev_hi, x_dev_lo, Wr, W1, b1, W2, b2, split, wmats,
               logits=None):
    """One MoE level: host routing + device batched-expert FFN + host combine.
    xf: [ntok, D] f32 level input (for the affine terms; routing uses
    `logits` when given, else xf @ Wr). x_dev_hi/lo: bf16 FFN input.
    Returns ([ntok, D] f32 level output, [ntok, E1] router correction or
    None)."""
    E = Wr.shape[1]
    p, w, idx = _route(xf, Wr, logits)
    caps, NTOT, perm, wslot, gid, gw = _make_slots(idx, w, E)
    nc = _build_ffn(E, caps, split)

    in_maps = []
    for c in range(NCORES):
        m = dict(wmats)
        sel = perm[c]
        m["xh"] = np.ascontiguousarray(x_dev_hi[sel].T)
        if split:
            m["xl"] = np.ascontiguousarray(x_dev_lo[sel].T)
        m["ws"] = wslot[c:c + 1]
        in_maps.append(m)
    _LAST_IN_MAPS[("ffn", E, tuple(caps), split)] = in_maps
    res = run_bass_kernel_spmd(nc, in_maps, core_ids=list(range(NCORES)))
    Y = np.concatenate([res.results[c]["outT"] for c in range(NCORES)],
                       axis=1)                          # [D, NCORES*NTOT]
    Cc = np.einsum('ef,efd->ed', np.maximum(b1, 0.0), W2) + b2
    out = p @ Cc + w @ (b2 - Cc)
    out += Y[:, gid[:, 0]].T
    out += Y[:, gid[:, 1]].T
    dlog = None
    if split:
        DL = np.concatenate([res.results[c]["dlT"] for c in range(NCORES)],
                            axis=1)                     # [E1, NCORES*NTOT]
        dlog = (DL[:, gid[:, 0]].T * gw[:, 0:1]
                + DL[:, gid[:, 1]].T * gw[:, 1:2])
    return out, dlog


def kernel(x, Wr0, W1_0, b1_0, W2_0, b2_0, Wr1, W1_1, b1_1, W2_1, b2_1,
           **extra):
    x = np.asarray(x, np.float32)
    B, S, _ = x.shape
    xf = np.ascontiguousarray(x.reshape(B * S, D))

    def hi_lo(a):
        h = np.asarray(a, np.float32).astype(BF)
        lo = (np.asarray(a, np.float32) - h.astype(np.float32)).astype(BF)
        return h, lo

    x_hi, x_lo = hi_lo(xf)
    w1h0, w1l0 = hi_lo(W1_0)
    W2_0f = np.asarray(W2_0, np.float32)
    w2h0 = W2_0f.astype(BF)
    Wr1f = np.asarray(Wr1, np.float32)
    vf = np.einsum('efd,dc->efc', W2_0f, Wr1f).astype(BF)
    vl = np.einsum('efd,dc->efc',
                   W2_0f - w2h0.astype(np.float32), Wr1f).astype(BF)
    wm0 = {"w1h": w1h0, "w1l": w1l0, "w2h": w2h0, "vf": vf, "vl": vl,
           "b1": np.ascontiguousarray(b1_0, np.float32)}
    b1_0f = np.ascontiguousarray(b1_0, np.float32)
    b2_0f = np.asarray(b2_0, np.float32)
    Wr0f = np.asarray(Wr0, np.float32)
    r = _run_level_seg(xf, x_hi, x_lo, Wr0f, W1_0, b1_0f, W2_0, b2_0f,
                       True, wm0)
    if r is None:
        r = _run_level(xf, x_hi, x_lo, Wr0f, W1_0, b1_0f, W2_0, b2_0f,
                       True, wm0)
    h0, dlog = r

    h0 = np.ascontiguousarray(h0, np.float32)
    logits1 = h0 @ Wr1f + dlog
    wm1 = {"w1h": np.asarray(W1_1, np.float32).astype(BF),
           "w2h": np.asarray(W2_1, np.float32).astype(BF),
           "b1": np.ascontiguousarray(b1_1, np.float32)}
    b1_1f = np.ascontiguousarray(b1_1, np.float32)
    b2_1f = np.asarray(b2_1, np.float32)
    r = _run_level_seg(h0, h0.astype(BF), None, Wr1f, W1_1, b1_1f, W2_1,
                       b2_1f, False, wm1, logits=logits1)
    if r is None:
        r = _run_level(h0, h0.astype(BF), None, Wr1f, W1_1, b1_1f, W2_1,
                       b2_1f, False, wm1, logits=logits1)
    out, _ = r
    return np.ascontiguousarray(out, np.float32).reshape(B, S, D)

